# revision 12
# baseline (speedup 1.0000x reference)
"""Causal self-attention (RoPE, 16 heads) on 8 TRN2 NeuronCores.

Sharding: core c handles batch b = c//2 and head half (c%2)*8..+8.
Each core computes a partial output projection (T, C); the host sums
the two partials per batch. No on-device collectives.

All matmuls run as float32r (full PE rate at N>=256, ~1.5e-4 rel err).
Softmax runs unnormalized with a ones-column appended to V (M=65 AV
matmuls produce y and the row-sum together); normalization is applied
after attention via reciprocal + partition broadcast.

v2: W_attn columns are pre-permuted so each QKV matmul emits a head
pair directly in the final [E|O] per-head partition layout; RoPE uses
a partner tile built by two partition-block-swap DMAs plus a
sign-baked sin table (no post-rope repack DMAs). Big loads are fused
into single DMAs (W on the gpsimd SWDGE queue to bypass HWDGE), the
ones column is memset, normalization writes partition-shifted straight
into the y staging tiles, and the projection DMAs straight from PSUM.
"""

import sys
from contextlib import ExitStack

import numpy as np

sys.path.insert(0, "/opt/trn_rl_repo")

import concourse.bacc as bacc
import concourse.mybir as mybir
import concourse.tile as tile
from concourse.bass_utils import run_bass_kernel_spmd

F32 = mybir.dt.float32
F32R = mybir.dt.float32r
EXP = mybir.ActivationFunctionType.Exp

# Problem constants
B, T, C = 4, 2048, 1024
H = 16          # global heads
D = 64          # head dim
HL = 8          # heads per core
N_CORES = 8
ROPE_BASE = 10000.0
SCALE = 1.0 / 8.0  # 1/sqrt(D)

# Derived tiling
TCH = 512            # T-chunk (q-chunk) width
NT = T // TCH        # 4 T-chunks
NJT = T // 128       # 16 k-tiles
CB = C // 128        # 8 contraction chunks
VW = D + 1           # V columns per head incl. ones column


def _build_program(tt=T):
    """Build the SPMD program for sequence length tt (tt % 512 == 0)."""
    nt = tt // TCH
    njt = tt // 128

    nc = bacc.Bacc("TRN2", target_bir_lowering=False, debug=False)
    xt = nc.dram_tensor("xt", (C, tt), F32R, kind="ExternalInput").ap()
    wqkv = nc.dram_tensor("wqkv", (C, 3 * C // 2), F32R, kind="ExternalInput").ap()
    wproj = nc.dram_tensor("wproj", (C // 2, C), F32R, kind="ExternalInput").ap()
    cosr = nc.dram_tensor("cosr", (128, tt), F32, kind="ExternalInput").ap()
    sinr = nc.dram_tensor("sinr", (128, tt), F32, kind="ExternalInput").ap()
    trimask = nc.dram_tensor("trimask", (128, 128), F32, kind="ExternalInput").ap()
    out = nc.dram_tensor("out", (tt, C), F32, kind="ExternalOutput").ap()

    xt_r = xt.rearrange("(cb p) t -> p cb t", p=128)       # [128, 8, tt]
    wqkv_r = wqkv.rearrange("(cb p) f -> p cb f", p=128)   # [128, 8, 1536]
    wproj_r = wproj.rearrange("(m p) o -> p m o", p=128)   # [128, 4, 1024]

    with tile.TileContext(nc) as tc, ExitStack() as ctx:
        # ---- persistent buffers ----
        persist = ctx.enter_context(tc.tile_pool(name="persist", bufs=1))
        qtr_all = persist.tile([128, 4 * tt], F32R, name="qtra", tag="qtra")
        ktr_all = persist.tile([128, 4 * tt], F32R, name="ktra", tag="ktra")
        qtr = [qtr_all[:, i * tt:(i + 1) * tt] for i in range(4)]
        ktr = [ktr_all[:, i * tt:(i + 1) * tt] for i in range(4)]
        vp = persist.tile([128, njt * VW * HL], F32R, name="vp", tag="vp")
        tri = persist.tile([128, 128], F32, name="tri", tag="tri")
        wp_t = persist.tile([128, 4, C], F32R, name="wp", tag="wp")

        nc.scalar.dma_start(out=tri, in_=trimask)
        nc.scalar.dma_start(out=wp_t, in_=wproj_r)

        # ones columns of Vp: col j*VW*HL + lh*VW + D for all j, lh
        vp_r = vp.rearrange("p (j lh w) -> p j lh w", j=njt, lh=HL)
        ones_t = persist.tile([128, njt * HL], F32, name="ones", tag="ones")
        nc.vector.memset(ones_t, 1.0)
        vp_n = vp.rearrange("p (n w) -> p n w", w=VW)
        nc.scalar.copy(vp_n[:, :, D:D + 1],
                       ones_t.rearrange("p (n w) -> p n w", w=1))

        # ================= Phase 1: QKV + RoPE =================
        with ExitStack() as p1:
            xt_pool = p1.enter_context(tc.tile_pool(name="xt", bufs=2))
            w_pool = p1.enter_context(tc.tile_pool(name="w", bufs=2))
            rp_pool = p1.enter_context(tc.tile_pool(name="rope", bufs=1))
            cssn_pool = p1.enter_context(tc.tile_pool(name="cssn", bufs=1))
            ps_qk = p1.enter_context(tc.tile_pool(name="psqk", bufs=2, space="PSUM"))
            ps_v = p1.enter_context(tc.tile_pool(name="psv", bufs=2, space="PSUM"))

            cs_t = cssn_pool.tile([128, tt], F32, name="cs", tag="cs")
            sn_t = cssn_pool.tile([128, tt], F32, name="sn", tag="sn")
            nc.scalar.dma_start(out=cs_t, in_=cosr)
            nc.scalar.dma_start(out=sn_t, in_=sinr)

            def qk_mms(tci, x_t, w_t):
                # two head-pair chunks, each already in final [E|O] layout,
                # fused into one PSUM tile [128, k(2), TCH]
                ps12 = ps_qk.tile([128, 2, TCH], F32, name="ps12", tag="ps12")
                for k in range(2):
                    for cb in range(CB):
                        nc.tensor.matmul(
                            ps12[:, k], w_t[:, cb, 128 * k:128 * (k + 1)],
                            x_t[:, cb, :],
                            start=(cb == 0), stop=(cb == CB - 1))
                return ps12

            def qk_rope(fg, grp, pss):
                # a = ps*cos, b = ps*(sign-baked sin); block-swapping b
                # (E<->O 32-blocks) gives out = a + swap(b) directly in the
                # final q/k chunk layout. b for the whole fg-call lives in
                # one [128, 2048] tile so the swap is 4 plain-sliced DMAs.
                dst_all = qtr_all if fg < 2 else ktr_all
                c0 = 2 * (fg % 2)
                tb = rp_pool.tile([128, 2, 2, TCH], F32, name="tb")
                bs = rp_pool.tile([128, 2, 2, TCH], F32, name="bs")
                tas = []
                for ti, tci in enumerate(grp):
                    tsl = slice(tci * TCH, (tci + 1) * TCH)
                    ta = rp_pool.tile([128, 2, TCH], F32, name=f"ta{ti}")
                    for k in range(2):
                        nc.vector.tensor_mul(ta[:, k], pss[ti][:, k],
                                             cs_t[:, tsl])
                        nc.vector.tensor_mul(tb[:, ti, k], pss[ti][:, k],
                                             sn_t[:, tsl])
                    tas.append(ta)
                tbf = tb.rearrange("p a b t -> p (a b t)")
                bsf = bs.rearrange("p a b t -> p (a b t)")
                nc.sync.dma_start(out=bsf[0:32, :], in_=tbf[32:64, :])
                nc.gpsimd.dma_start(out=bsf[32:64, :], in_=tbf[0:32, :])
                nc.sync.dma_start(out=bsf[64:96, :], in_=tbf[96:128, :])
                nc.gpsimd.dma_start(out=bsf[96:128, :], in_=tbf[64:96, :])
                dv = dst_all.rearrange("p (c t) -> p c t", c=4)
                for ti, tci in enumerate(grp):
                    tsl = slice(tci * TCH, (tci + 1) * TCH)
                    nc.vector.tensor_add(dv[:, c0:c0 + 2, tsl], tas[ti],
                                         bs[:, ti])

            def v_block(vh, tci, x_t, w_t):
                for tt4 in range(4):
                    jt = tci * 4 + tt4
                    ps_vv = ps_v.tile([128, 256], F32, name="psvv", tag="psvv")
                    for cb in range(CB):
                        nc.tensor.matmul(
                            ps_vv,
                            x_t[:, cb, tt4 * 128:(tt4 + 1) * 128],
                            w_t[:, cb, :],
                            start=(cb == 0), stop=(cb == CB - 1))
                    # copy into Vp: 4 heads (vh*4..vh*4+3), 64 cols each,
                    # strided by VW to leave the ones column
                    dstv = vp_r[:, jt, vh * 4:vh * 4 + 4, 0:D]
                    nc.scalar.copy(dstv, ps_vv.rearrange(
                        "p (h d) -> p h d", h=4))

            def load_w(col0, split_first=False):
                w_t = w_pool.tile([128, CB, 256], F32R, name="wtile")
                if split_first:
                    nc.gpsimd.dma_start(out=w_t[:, 0, :],
                                        in_=wqkv_r[:, 0, col0:col0 + 256])
                    nc.gpsimd.dma_start(out=w_t[:, 1:, :],
                                        in_=wqkv_r[:, 1:, col0:col0 + 256])
                else:
                    nc.gpsimd.dma_start(out=w_t,
                                        in_=wqkv_r[:, :, col0:col0 + 256])
                return w_t

            def load_xt(tci, nsplit=1):
                x_t = xt_pool.tile([128, CB, TCH], F32R, name="xtile")
                tsl = slice(tci * TCH, (tci + 1) * TCH)
                if nsplit == 1:
                    nc.sync.dma_start(out=x_t, in_=xt_r[:, :, tsl])
                else:
                    bounds = [0, 1, 4, CB]
                    for lo, hi in zip(bounds[:-1], bounds[1:]):
                        nc.sync.dma_start(out=x_t[:, lo:hi, :],
                                          in_=xt_r[:, lo:hi, tsl])
                return x_t

            # T-pairs share one load of W (halves W traffic)
            for g0 in range(0, nt, 2):
                grp = list(range(g0, min(g0 + 2, nt)))
                xts = {grp[0]: load_xt(grp[0], nsplit=3 if g0 == 0 else 1)}
                w_next = load_w(0, split_first=(g0 == 0))
                for tci in grp[1:]:
                    xts[tci] = load_xt(tci)
                # fg 0,1 -> Q chunk pairs (0,1), (2,3); fg 2,3 -> K
                for fg in range(4):
                    w_t = w_next
                    pss = [qk_mms(tci, xts[tci], w_t) for tci in grp]
                    w_next = load_w(256 * (fg + 1))  # prefetch
                    qk_rope(fg, grp, pss)
                # V in natural [t, f] layout, 2 halves of 256 features
                for vh in range(2):
                    w_t = w_next
                    if vh == 0:
                        w_next = load_w(C + 256)
                    for tci in grp:
                        v_block(vh, tci, xts[tci], w_t)

        # ================= Phase 2: attention + proj =================
        with ExitStack() as p2:
            y_pool = p2.enter_context(tc.tile_pool(name="ytnp", bufs=1))
            e_pool = p2.enter_context(tc.tile_pool(name="expt", bufs=3))
            n_pool = p2.enter_context(tc.tile_pool(name="norm", bufs=2))
            o_pool = p2.enter_context(tc.tile_pool(name="outc", bufs=2))
            ps_s = p2.enter_context(tc.tile_pool(name="pss", bufs=2, space="PSUM"))
            ps_y = p2.enter_context(tc.tile_pool(name="psy", bufs=4, space="PSUM"))

            ytn = [y_pool.tile([128, tt], F32R, name=f"ytn{i}", tag=f"ytn{i}")
                   for i in range(4)]

            def emit_proj(qc, tt4s=range(4)):
                qbase = qc * TCH
                for tt4 in tt4s:
                    tsl = slice(qbase + tt4 * 128, qbase + (tt4 + 1) * 128)
                    ot = o_pool.tile([128, C], F32, name="ot")
                    for oc in range(2):
                        pp = ps_y.tile([128, TCH], F32, name="pp", tag="y")
                        for m in range(4):
                            nc.tensor.matmul(
                                pp, ytn[m][:, tsl],
                                wp_t[:, m, oc * TCH:(oc + 1) * TCH],
                                start=(m == 0), stop=(m == 3))
                        nc.vector.tensor_copy(
                            ot[:, oc * TCH:(oc + 1) * TCH], pp)
                    nc.sync.dma_start(out=out[tsl, :], in_=ot)

            for qc in range(nt):
                qbase = qc * TCH
                for pr in range(4):
                    h_a, h_b = 2 * pr, 2 * pr + 1
                    y_a = ps_y.tile([D + 1, TCH], F32, name="ya", tag="y")
                    y_b = ps_y.tile([D + 1, TCH], F32, name="yb", tag="y")
                    njs = 4 * qc + 4
                    for j in range(njs):
                        r = j - 4 * qc
                        q0 = 128 * max(r, 0)
                        qsl = slice(qbase + q0, qbase + TCH)
                        jsl = slice(j * 128, (j + 1) * 128)
                        s_ab = ps_s.tile([128, 2 * TCH], F32, name="sab", tag="sab")
                        nc.tensor.matmul(
                            s_ab[:, q0:TCH],
                            ktr[pr][0:64, jsl],
                            qtr[pr][0:64, qsl],
                            start=True, stop=True)
                        nc.tensor.matmul(
                            s_ab[:, TCH + q0:2 * TCH],
                            ktr[pr][64:128, jsl],
                            qtr[pr][64:128, qsl],
                            start=True, stop=True)
                        e_ab = e_pool.tile([128, 2 * TCH], F32R, name="eab")
                        if q0 == 0:
                            nc.scalar.activation(e_ab, s_ab, EXP, scale=SCALE)
                        else:
                            nc.scalar.activation(e_ab[:, q0:TCH],
                                                 s_ab[:, q0:TCH], EXP,
                                                 scale=SCALE)
                            nc.scalar.activation(e_ab[:, TCH + q0:2 * TCH],
                                                 s_ab[:, TCH + q0:2 * TCH], EXP,
                                                 scale=SCALE)
                        if r >= 0:
                            nc.vector.tensor_mul(
                                e_ab[:, q0:q0 + 128], e_ab[:, q0:q0 + 128], tri)
                            nc.vector.tensor_mul(
                                e_ab[:, TCH + q0:TCH + q0 + 128],
                                e_ab[:, TCH + q0:TCH + q0 + 128], tri)
                        vb = j * VW * HL
                        nc.tensor.matmul(
                            y_a[:, q0:TCH],
                            vp[:, vb + h_a * VW:vb + (h_a + 1) * VW],
                            e_ab[:, q0:TCH],
                            start=(j == 0), stop=(j == njs - 1))
                        nc.tensor.matmul(
                            y_b[:, q0:TCH],
                            vp[:, vb + h_b * VW:vb + (h_b + 1) * VW],
                            e_ab[:, TCH + q0:2 * TCH],
                            start=(j == 0), stop=(j == njs - 1))
                    if qc > 0:
                        # deferred projection of the previous q-chunk, one
                        # T-tile per pair, right after the j-loop: the PE
                        # fills the ACT tail and reuses freed y slots
                        emit_proj(qc - 1, [pr])
                    for h, y_t in ((h_a, y_a), (h_b, y_b)):
                        rt0 = n_pool.tile([1, TCH], F32, name=f"rt{h % 2}")
                        nc.vector.reciprocal(rt0, y_t[D:D + 1, :])
                        rb = n_pool.tile([D, TCH], F32, name=f"rb{h % 2}")
                        nc.gpsimd.partition_broadcast(rb, rt0)
                        nc.vector.tensor_mul(
                            ytn[h // 2][64 * (h % 2):64 * (h % 2) + 64,
                                        qbase:qbase + TCH],
                            y_t[0:D, :], rb)
            emit_proj(nt - 1)

    nc.compile()
    return nc


def _prep_inputs(x, w_attn, w_proj, tt=T):
    """Per-core host sharding. Returns in_maps list."""
    x = np.asarray(x, dtype=np.float32)
    w_attn = np.asarray(w_attn, dtype=np.float32)
    w_proj = np.asarray(w_proj, dtype=np.float32)

    # rope tables [128, tt]: row p -> pair index p % 32; sin rows carry the
    # rope sign: -1 on even (E) 32-blocks, +1 on odd (O) blocks
    ip = np.arange(128) % 32
    inv = ROPE_BASE ** (-(2.0 * ip) / D)
    t_idx = np.arange(tt, dtype=np.float64)
    ang = t_idx[None, :] * inv[:, None]
    cosr = np.cos(ang).astype(np.float32)
    sgn = np.where((np.arange(128) // 32) % 2 == 0, 1.0, -1.0)
    sinr = (np.sin(ang) * sgn[:, None]).astype(np.float32)

    k_idx = np.arange(128)
    q_idx = np.arange(128)
    trimask = (k_idx[:, None] <= q_idx[None, :]).astype(np.float32)

    in_maps = []
    for c in range(N_CORES):
        b = c // 2
        hb = (c % 2) * HL
        # Q/K column permutation: chunk-pair-major, head, then E dims (2i)
        # followed by O dims (2i+1) -- matmul output lands in final layout
        cols = []
        for qk in range(2):  # 0=Q, 1=K
            for ch in range(4):
                for h2 in range(2):
                    h = hb + 2 * ch + h2
                    for par in range(2):  # 0=even dims, 1=odd dims
                        for i in range(32):
                            cols.append(qk * C + h * D + 2 * i + par)
        for lh in range(HL):  # V natural
            h = hb + lh
            for d in range(D):
                cols.append(2 * C + h * D + d)
        wqkv_c = np.ascontiguousarray(w_attn[:, cols])
        wproj_c = np.ascontiguousarray(w_proj[hb * D:(hb + HL) * D, :])
        xt_c = np.ascontiguousarray(x[b, :tt].T)
        in_maps.append({
            "xt": xt_c, "wqkv": wqkv_c, "wproj": wproj_c,
            "cosr": cosr, "sinr": sinr, "trimask": trimask,
        })
    return in_maps


_PROGRAM_CACHE = {}


def _get_program(tt=T):
    if tt not in _PROGRAM_CACHE:
        _PROGRAM_CACHE[tt] = _build_program(tt)
    return _PROGRAM_CACHE[tt]


def run(x, w_attn, w_proj, tt=T, **run_kwargs):
    nc = _get_program(tt)
    in_maps = _prep_inputs(x, w_attn, w_proj, tt)
    res = run_bass_kernel_spmd(nc, in_maps, core_ids=list(range(N_CORES)),
                               **run_kwargs)
    parts = [res.results[c]["out"] for c in range(N_CORES)]
    y = np.stack([parts[2 * b] + parts[2 * b + 1] for b in range(B)])
    return y, res


def kernel(x, W_attn, W_proj):
    y, _ = run(x, W_attn, W_proj, tt=T)
    return y


# revision 19
# speedup vs baseline: 1.3004x; 1.3004x over previous
"""Causal self-attention (RoPE, 16 heads) on 8 TRN2 NeuronCores.

Sharding: core c handles batch b = c//2 and head half (c%2)*8..+8.
Each core computes a partial output projection (T, C); the host sums
the two partials per batch. No on-device collectives.

All matmuls run as float32r (full PE rate at N>=256, ~1.5e-4 rel err).
Softmax runs unnormalized with a ones-column appended to V (M=65 AV
matmuls produce y and the row-sum together); normalization is applied
after attention via reciprocal + partition broadcast.

v2: W_attn columns are pre-permuted so each QKV matmul emits a head
pair directly in the final [E|O] per-head partition layout; RoPE uses
a partner tile built by two partition-block-swap DMAs plus a
sign-baked sin table (no post-rope repack DMAs). Big loads are fused
into single DMAs (W on the gpsimd SWDGE queue to bypass HWDGE), the
ones column is memset, normalization writes partition-shifted straight
into the y staging tiles, and the projection DMAs straight from PSUM.
"""

import sys
from contextlib import ExitStack

import numpy as np

sys.path.insert(0, "/opt/trn_rl_repo")

import concourse.bacc as bacc
import concourse.mybir as mybir
import concourse.tile as tile
from concourse.bass_utils import run_bass_kernel_spmd

F32 = mybir.dt.float32
F32R = mybir.dt.float32r
BF16 = mybir.dt.bfloat16
EXP = mybir.ActivationFunctionType.Exp

# Problem constants
B, T, C = 4, 2048, 1024
H = 16          # global heads
D = 64          # head dim
HL = 8          # heads per core
N_CORES = 8
ROPE_BASE = 10000.0
SCALE = 1.0 / 8.0  # 1/sqrt(D)

# Derived tiling
TCH = 512            # T-chunk (q-chunk) width
NT = T // TCH        # 4 T-chunks
NJT = T // 128       # 16 k-tiles
CB = C // 128        # 8 contraction chunks
VW = D + 1           # V columns per head incl. ones column


def _build_program(tt=T):
    """Build the SPMD program for sequence length tt (tt % 512 == 0)."""
    nt = tt // TCH
    njt = tt // 128

    nc = bacc.Bacc("TRN2", target_bir_lowering=False, debug=False)
    xt = nc.dram_tensor("xt", (C, tt), F32R, kind="ExternalInput").ap()
    wqkv = nc.dram_tensor("wqkv", (C, 3 * C // 2), F32R, kind="ExternalInput").ap()
    wproj = nc.dram_tensor("wproj", (C // 2, C), F32R, kind="ExternalInput").ap()
    cosr = nc.dram_tensor("cosr", (128, tt), F32, kind="ExternalInput").ap()
    sinr = nc.dram_tensor("sinr", (128, tt), F32, kind="ExternalInput").ap()
    trimask = nc.dram_tensor("trimask", (128, 128), F32, kind="ExternalInput").ap()
    out = nc.dram_tensor("out", (tt, C), F32, kind="ExternalOutput").ap()

    xt_r = xt.rearrange("(cb p) t -> p cb t", p=128)       # [128, 8, tt]
    wqkv_r = wqkv.rearrange("(cb p) f -> p cb f", p=128)   # [128, 8, 1536]
    wproj_r = wproj.rearrange("(m p) o -> p m o", p=128)   # [128, 4, 1024]

    with tile.TileContext(nc) as tc, ExitStack() as ctx:
        # ---- persistent buffers ----
        persist = ctx.enter_context(tc.tile_pool(name="persist", bufs=1))
        qtr_all = persist.tile([128, 4 * tt], BF16, name="qtra", tag="qtra")
        ktr_all = persist.tile([128, 4 * tt], BF16, name="ktra", tag="ktra")
        qtr = [qtr_all[:, i * tt:(i + 1) * tt] for i in range(4)]
        ktr = [ktr_all[:, i * tt:(i + 1) * tt] for i in range(4)]
        vp = persist.tile([128, njt * VW * HL], F32R, name="vp", tag="vp")
        tri = persist.tile([128, 128], F32, name="tri", tag="tri")

        nc.scalar.dma_start(out=tri, in_=trimask)

        # ones columns of Vp: col j*VW*HL + lh*VW + D for all j, lh
        vp_r = vp.rearrange("p (j lh w) -> p j lh w", j=njt, lh=HL)
        ones_t = persist.tile([128, njt * HL], F32, name="ones", tag="ones")
        nc.vector.memset(ones_t, 1.0)
        vp_n = vp.rearrange("p (n w) -> p n w", w=VW)
        nc.scalar.copy(vp_n[:, :, D:D + 1],
                       ones_t.rearrange("p (n w) -> p n w", w=1))

        # ================= Phase 1: QKV + RoPE =================
        with ExitStack() as p1:
            xt_pool = p1.enter_context(tc.tile_pool(name="xt", bufs=3))
            w_pool = p1.enter_context(tc.tile_pool(name="w", bufs=3))
            rp_pool = p1.enter_context(tc.tile_pool(name="rope", bufs=2))
            cssn_pool = p1.enter_context(tc.tile_pool(name="cssn", bufs=1))
            ps_qk = p1.enter_context(tc.tile_pool(name="psqk", bufs=3, space="PSUM"))
            ps_v = p1.enter_context(tc.tile_pool(name="psv", bufs=2, space="PSUM"))

            cs_t = cssn_pool.tile([128, tt], F32, name="cs", tag="cs")
            sn_t = cssn_pool.tile([128, tt], F32, name="sn", tag="sn")
            nc.scalar.dma_start(out=cs_t, in_=cosr)
            nc.scalar.dma_start(out=sn_t, in_=sinr)

            def qk_mms(tci, x_t, w_t):
                # two head-pair chunks, each already in final [E|O] layout,
                # fused into one PSUM tile [128, k(2), TCH]
                ps12 = ps_qk.tile([128, 2, TCH], F32, name="ps12", tag="ps12")
                for k in range(2):
                    for cb in range(CB):
                        nc.tensor.matmul(
                            ps12[:, k], w_t[:, cb, 128 * k:128 * (k + 1)],
                            x_t[:, cb, :],
                            start=(cb == 0), stop=(cb == CB - 1))
                return ps12

            def rope_front(fg, grp, pss):
                # a = ps*cos, b = ps*(sign-baked sin); block-swapping b
                # (E<->O 32-blocks) gives out = a + swap(b) directly in the
                # final q/k chunk layout. b for the whole fg-call lives in
                # one [128, 2048] tile so the swap is 4 plain-sliced DMAs.
                tb = rp_pool.tile([128, 2, 2, TCH], BF16, name="tb")
                bs = rp_pool.tile([128, 2, 2, TCH], BF16, name="bs")
                tas = []
                for ti, tci in enumerate(grp):
                    tsl = slice(tci * TCH, (tci + 1) * TCH)
                    ta = rp_pool.tile([128, 2, TCH], BF16, name=f"ta{ti}")
                    for k in range(2):
                        nc.vector.tensor_mul(ta[:, k], pss[ti][:, k],
                                             cs_t[:, tsl])
                        nc.vector.tensor_mul(tb[:, ti, k], pss[ti][:, k],
                                             sn_t[:, tsl])
                    tas.append(ta)
                tbf = tb.rearrange("p a b t -> p (a b t)")
                bsf = bs.rearrange("p a b t -> p (a b t)")
                nc.sync.dma_start(out=bsf[0:32, :], in_=tbf[32:64, :])
                nc.gpsimd.dma_start(out=bsf[32:64, :], in_=tbf[0:32, :])
                nc.sync.dma_start(out=bsf[64:96, :], in_=tbf[96:128, :])
                nc.gpsimd.dma_start(out=bsf[96:128, :], in_=tbf[64:96, :])
                return (fg, grp, tas, bs)

            def rope_back(st):
                # deferred combine: runs one fg-step later so the swap DMA
                # latency never parks the DVE queue
                if st is None:
                    return
                fg, grp, tas, bs = st
                dst_all = qtr_all if fg < 2 else ktr_all
                c0 = 2 * (fg % 2)
                dv = dst_all.rearrange("p (c t) -> p c t", c=4)
                for ti, tci in enumerate(grp):
                    tsl = slice(tci * TCH, (tci + 1) * TCH)
                    nc.vector.tensor_add(dv[:, c0:c0 + 2, tsl], tas[ti],
                                         bs[:, ti])

            def v_block(vh, tci, x_t, w_t):
                for tt4 in range(4):
                    jt = tci * 4 + tt4
                    ps_vv = ps_v.tile([128, 256], F32, name="psvv", tag="psvv")
                    for cb in range(CB):
                        nc.tensor.matmul(
                            ps_vv,
                            x_t[:, cb, tt4 * 128:(tt4 + 1) * 128],
                            w_t[:, cb, :],
                            start=(cb == 0), stop=(cb == CB - 1))
                    # copy into Vp: 4 heads (vh*4..vh*4+3), 64 cols each,
                    # strided by VW to leave the ones column
                    dstv = vp_r[:, jt, vh * 4:vh * 4 + 4, 0:D]
                    nc.scalar.copy(dstv, ps_vv.rearrange(
                        "p (h d) -> p h d", h=4))

            def load_w(col0, split_first=False):
                w_t = w_pool.tile([128, CB, 256], F32R, name="wtile")
                if split_first:
                    nc.gpsimd.dma_start(out=w_t[:, 0:2, :],
                                        in_=wqkv_r[:, 0:2, col0:col0 + 256])
                    nc.gpsimd.dma_start(out=w_t[:, 2:, :],
                                        in_=wqkv_r[:, 2:, col0:col0 + 256])
                else:
                    nc.gpsimd.dma_start(out=w_t,
                                        in_=wqkv_r[:, :, col0:col0 + 256])
                return w_t

            def load_xt(tci, percb=False):
                x_t = xt_pool.tile([128, CB, TCH], F32R, name="xtile")
                tsl = slice(tci * TCH, (tci + 1) * TCH)
                if percb:
                    for cb in range(CB):
                        nc.sync.dma_start(out=x_t[:, cb, :],
                                          in_=xt_r[:, cb, tsl])
                else:
                    nc.sync.dma_start(out=x_t, in_=xt_r[:, :, tsl])
                return x_t

            # T-pairs share one load of W (halves W traffic); x/W for the
            # next group prefetch during the V phase of the current one
            xts = {0: load_xt(0, percb=True)}
            w_next = load_w(0, split_first=True)
            xts[1] = load_xt(1, percb=True)
            rope_st = None
            for g0 in range(0, nt, 2):
                grp = list(range(g0, min(g0 + 2, nt)))
                # fg 0,1 -> Q chunk pairs (0,1), (2,3); fg 2,3 -> K
                for fg in range(4):
                    w_t = w_next
                    pss = [qk_mms(tci, xts[tci], w_t) for tci in grp]
                    w_next = load_w(256 * (fg + 1))  # prefetch
                    st = rope_front(fg, grp, pss)
                    rope_back(rope_st)
                    rope_st = st
                # V in natural [t, f] layout, 2 halves of 256 features
                for vh in range(2):
                    w_t = w_next
                    if vh == 0:
                        w_next = load_w(C + 256)
                    elif g0 + 2 < nt:
                        w_next = load_w(0)
                    for tci in grp:
                        v_block(vh, tci, xts[tci], w_t)
                    if vh == 0:
                        rope_back(rope_st)
                        rope_st = None
                        if g0 + 2 < nt:
                            xts[g0 + 2] = load_xt(g0 + 2)
                            xts[g0 + 3] = load_xt(g0 + 3)

        # ================= Phase 2: attention + proj =================
        with ExitStack() as p2:
            y_pool = p2.enter_context(tc.tile_pool(name="ytnp", bufs=1))
            e_pool = p2.enter_context(tc.tile_pool(name="expt", bufs=3))
            n_pool = p2.enter_context(tc.tile_pool(name="norm", bufs=2))
            o_pool = p2.enter_context(tc.tile_pool(name="outc", bufs=2))
            ps_s = p2.enter_context(tc.tile_pool(name="pss", bufs=2, space="PSUM"))
            ps_y = p2.enter_context(tc.tile_pool(name="psy", bufs=4, space="PSUM"))

            ytn = [y_pool.tile([128, tt], F32R, name=f"ytn{i}", tag=f"ytn{i}")
                   for i in range(4)]
            wp_t = y_pool.tile([128, 4, C], F32R, name="wp", tag="wp")
            nc.sync.dma_start(out=wp_t, in_=wproj_r)

            def emit_proj(qc, tt4s=range(4)):
                qbase = qc * TCH
                for tt4 in tt4s:
                    tsl = slice(qbase + tt4 * 128, qbase + (tt4 + 1) * 128)
                    ot = o_pool.tile([128, C], F32, name="ot")
                    for oc in range(2):
                        pp = ps_y.tile([128, TCH], F32, name="pp", tag="y")
                        for m in range(4):
                            nc.tensor.matmul(
                                pp, ytn[m][:, tsl],
                                wp_t[:, m, oc * TCH:(oc + 1) * TCH],
                                start=(m == 0), stop=(m == 3))
                        nc.vector.tensor_copy(
                            ot[:, oc * TCH:(oc + 1) * TCH], pp)
                    nc.sync.dma_start(out=out[tsl, :], in_=ot)

            for qc in range(nt):
                qbase = qc * TCH
                for pr in range(4):
                    h_a, h_b = 2 * pr, 2 * pr + 1
                    y_a = ps_y.tile([D + 1, TCH], F32, name="ya", tag="y")
                    y_b = ps_y.tile([D + 1, TCH], F32, name="yb", tag="y")
                    njs = 4 * qc + 4
                    for j in range(njs):
                        r = j - 4 * qc
                        q0 = 128 * max(r, 0)
                        qsl = slice(qbase + q0, qbase + TCH)
                        jsl = slice(j * 128, (j + 1) * 128)
                        s_ab = ps_s.tile([128, 2 * TCH], F32, name="sab", tag="sab")
                        nc.tensor.matmul(
                            s_ab[:, q0:TCH],
                            ktr[pr][0:64, jsl],
                            qtr[pr][0:64, qsl],
                            start=True, stop=True)
                        nc.tensor.matmul(
                            s_ab[:, TCH + q0:2 * TCH],
                            ktr[pr][64:128, jsl],
                            qtr[pr][64:128, qsl],
                            start=True, stop=True)
                        e_ab = e_pool.tile([128, 2 * TCH], F32R, name="eab")
                        if q0 == 0:
                            nc.scalar.activation(e_ab, s_ab, EXP, scale=SCALE)
                        else:
                            nc.scalar.activation(e_ab[:, q0:TCH],
                                                 s_ab[:, q0:TCH], EXP,
                                                 scale=SCALE)
                            nc.scalar.activation(e_ab[:, TCH + q0:2 * TCH],
                                                 s_ab[:, TCH + q0:2 * TCH], EXP,
                                                 scale=SCALE)
                        if r >= 0:
                            nc.vector.tensor_mul(
                                e_ab[:, q0:q0 + 128], e_ab[:, q0:q0 + 128], tri)
                            nc.vector.tensor_mul(
                                e_ab[:, TCH + q0:TCH + q0 + 128],
                                e_ab[:, TCH + q0:TCH + q0 + 128], tri)
                        vb = j * VW * HL
                        nc.tensor.matmul(
                            y_a[:, q0:TCH],
                            vp[:, vb + h_a * VW:vb + (h_a + 1) * VW],
                            e_ab[:, q0:TCH],
                            start=(j == 0), stop=(j == njs - 1))
                        nc.tensor.matmul(
                            y_b[:, q0:TCH],
                            vp[:, vb + h_b * VW:vb + (h_b + 1) * VW],
                            e_ab[:, TCH + q0:2 * TCH],
                            start=(j == 0), stop=(j == njs - 1))
                    if qc > 0:
                        # deferred projection of the previous q-chunk, one
                        # T-tile per pair, right after the j-loop: the PE
                        # fills the ACT tail and reuses freed y slots
                        emit_proj(qc - 1, [pr])
                    for h, y_t in ((h_a, y_a), (h_b, y_b)):
                        rt0 = n_pool.tile([1, TCH], F32, name=f"rt{h % 2}")
                        nc.vector.reciprocal(rt0, y_t[D:D + 1, :])
                        rb = n_pool.tile([D, TCH], F32, name=f"rb{h % 2}")
                        nc.gpsimd.partition_broadcast(rb, rt0)
                        nc.vector.tensor_mul(
                            ytn[h // 2][64 * (h % 2):64 * (h % 2) + 64,
                                        qbase:qbase + TCH],
                            y_t[0:D, :], rb)
            emit_proj(nt - 1)

    nc.compile()
    return nc


def _prep_inputs(x, w_attn, w_proj, tt=T):
    """Per-core host sharding. Returns in_maps list."""
    x = np.asarray(x, dtype=np.float32)
    w_attn = np.asarray(w_attn, dtype=np.float32)
    w_proj = np.asarray(w_proj, dtype=np.float32)

    # rope tables [128, tt]: row p -> pair index p % 32; sin rows carry the
    # rope sign: -1 on even (E) 32-blocks, +1 on odd (O) blocks
    ip = np.arange(128) % 32
    inv = ROPE_BASE ** (-(2.0 * ip) / D)
    t_idx = np.arange(tt, dtype=np.float64)
    ang = t_idx[None, :] * inv[:, None]
    cosr = np.cos(ang).astype(np.float32)
    sgn = np.where((np.arange(128) // 32) % 2 == 0, 1.0, -1.0)
    sinr = (np.sin(ang) * sgn[:, None]).astype(np.float32)

    k_idx = np.arange(128)
    q_idx = np.arange(128)
    trimask = (k_idx[:, None] <= q_idx[None, :]).astype(np.float32)

    in_maps = []
    for c in range(N_CORES):
        b = c // 2
        hb = (c % 2) * HL
        # Q/K column permutation: chunk-pair-major, head, then E dims (2i)
        # followed by O dims (2i+1) -- matmul output lands in final layout
        cols = []
        for qk in range(2):  # 0=Q, 1=K
            for ch in range(4):
                for h2 in range(2):
                    h = hb + 2 * ch + h2
                    for par in range(2):  # 0=even dims, 1=odd dims
                        for i in range(32):
                            cols.append(qk * C + h * D + 2 * i + par)
        for lh in range(HL):  # V natural
            h = hb + lh
            for d in range(D):
                cols.append(2 * C + h * D + d)
        wqkv_c = np.ascontiguousarray(w_attn[:, cols])
        wproj_c = np.ascontiguousarray(w_proj[hb * D:(hb + HL) * D, :])
        xt_c = np.ascontiguousarray(x[b, :tt].T)
        in_maps.append({
            "xt": xt_c, "wqkv": wqkv_c, "wproj": wproj_c,
            "cosr": cosr, "sinr": sinr, "trimask": trimask,
        })
    return in_maps


_PROGRAM_CACHE = {}


def _get_program(tt=T):
    if tt not in _PROGRAM_CACHE:
        _PROGRAM_CACHE[tt] = _build_program(tt)
    return _PROGRAM_CACHE[tt]


def run(x, w_attn, w_proj, tt=T, **run_kwargs):
    nc = _get_program(tt)
    in_maps = _prep_inputs(x, w_attn, w_proj, tt)
    res = run_bass_kernel_spmd(nc, in_maps, core_ids=list(range(N_CORES)),
                               **run_kwargs)
    parts = [res.results[c]["out"] for c in range(N_CORES)]
    y = np.stack([parts[2 * b] + parts[2 * b + 1] for b in range(B)])
    return y, res


def kernel(x, W_attn, W_proj):
    y, _ = run(x, W_attn, W_proj, tt=T)
    return y
